# revision 21
# baseline (speedup 1.0000x reference)
"""Trainium2 Bass kernel for nn_CausalAttention_41961830482398.

Computes, for H,T [8192,512] and dim-512 linear layers Wq/Wk/Wv/Wo:
    dist  = pairwise_distances(T)                 # [N,N]
    scale = 1/(1 + mean(dist, axis=1))            # [N,1]
    Q,K,V = H@W{q,k,v}.T + b{q,k,v}
    attn  = softmax(Q@K.T / sqrt(512))
    out   = ((attn*scale) @ V) @ Wo.T + bo

Sharding: sequence-parallel over the row dim N across 8 cores (1024 rows
per core).  Each core computes only its own K/V shard; the full K/V are
assembled with an AllGather that overlaps the distance phase.
Everything is computed in a transposed ("S^T") layout so that no
on-device transposes are needed:

  phase A: Kt_s = Wk@Hs^T + bk [512,1024], V_s = Hs@Wv^T + bv
           [1024,512], Qt = Wq@Hs^T + bq [512,1024]; AllGather Kt/V.
  phase B: G = Ts@T^T, dist = sqrt(max(tts+ttn-2G,0)+1e-8),
           row-mean via ACT accum -> scale_s  [1024]
           (overlaps the K/V AllGather)
  phase C: S^T tile = K@Qs^T  [n=128, m=512]; Pt = exp(S^T/sqrt(d));
           O^T += V^T@P^T via lhsT=V tiles; softmax denominator
           accumulated on DVE + one fp32 ones-matmul partition-reduce;
           projection: Out^T = (Wo@O^T)*(scale_s/denom) + bo -> DRAM.

Host passes pre-transposed/bf16-cast inputs and per-core shard slices;
the kernel returns Out^T per core which the host gathers + transposes.
"""

import numpy as np
import ml_dtypes

import concourse.bass as bass
import concourse.mybir as mybir
import concourse.tile as tile
from concourse import bacc
from concourse import bass_utils

N, DIM = 8192, 512
NCORES = 8
SHARD = N // NCORES          # 1024 rows per core
DC = DIM // 128              # 4 contraction chunks
NT = N // 128                # 64 key tiles
MBS = 512                    # m free-dim block
MBN = SHARD // MBS           # 2 m-blocks
NCH = N // 512               # 16 n chunks of 512
MT = SHARD // 128            # 8 m tiles per core
INV_SQRT_D = 1.0 / np.sqrt(np.float32(DIM))

BF16 = mybir.dt.bfloat16
F32 = mybir.dt.float32
AF = mybir.ActivationFunctionType
ALU = mybir.AluOpType
AX = mybir.AxisListType

bf16np = ml_dtypes.bfloat16


def _bcast_rows(ap, p=128):
    """DRAM row [1, n] -> broadcast AP [[0,p], [1,n]] for DMA replication."""
    return bass.AP(tensor=ap.tensor, offset=ap.offset, ap=[[0, p]] + ap.ap[1:])


def build_kernel():
    nc = bacc.Bacc("TRN2", target_bir_lowering=False, debug=False)

    # ---- DRAM I/O ------------------------------------------------------
    Hts = nc.dram_tensor("Hts", [DIM, SHARD], BF16, kind="ExternalInput")
    Tt = nc.dram_tensor("Tt", [DIM, N], BF16, kind="ExternalInput")
    Tts = nc.dram_tensor("Tts", [DIM, SHARD], BF16, kind="ExternalInput")
    ttn = nc.dram_tensor("ttn", [1, N], F32, kind="ExternalInput")
    tts = nc.dram_tensor("tts", [128, MT], F32, kind="ExternalInput")
    WqT = nc.dram_tensor("WqT", [DIM, DIM], BF16, kind="ExternalInput")
    WkT = nc.dram_tensor("WkT", [DIM, DIM], BF16, kind="ExternalInput")
    WvT = nc.dram_tensor("WvT", [DIM, DIM], BF16, kind="ExternalInput")
    WoT = nc.dram_tensor("WoT", [DIM, DIM], BF16, kind="ExternalInput")
    bq_c = nc.dram_tensor("bq_c", [128, DC], F32, kind="ExternalInput")
    bk_c = nc.dram_tensor("bk_c", [128, DC], F32, kind="ExternalInput")
    bo_c = nc.dram_tensor("bo_c", [128, DC], F32, kind="ExternalInput")
    bv_r = nc.dram_tensor("bv_r", [1, DIM], F32, kind="ExternalInput")
    OutT = nc.dram_tensor("OutT", [DIM, SHARD], F32, kind="ExternalOutput")

    with tile.TileContext(nc) as tc:
        with (
            tc.tile_pool(name="dram", bufs=1, space="DRAM") as dpool,
            tc.tile_pool(name="singles", bufs=1) as sg,
        ):
            sscr = dpool.tile([MT, 128], F32, tag="sscr")
            oscr = dpool.tile([MBN, MBS], F32, tag="oscr")
            ktag_i = dpool.tile([DIM, SHARD], BF16, tag="ktagi")
            ktag_o = dpool.tile([NCORES, DIM, SHARD], BF16, tag="ktago",
                                addr_space="Shared")
            vag_i = dpool.tile([SHARD, DIM], BF16, tag="vagi")
            vag_o = dpool.tile([NCORES, SHARD, DIM], BF16, tag="vago",
                               addr_space="Shared")

            # SBUF-resident for the whole kernel (KB/partition in comments)
            V_all = sg.tile([128, NT, DIM], BF16, tag="v")       # 64
            Kt_all = sg.tile([128, DC, N], BF16, tag="kt")       # 64
            Qt_all = sg.tile([128, DC, SHARD], BF16, tag="qt")   # 8
            WoT_all = sg.tile([128, DC, DIM], BF16, tag="wot")   # 4
            bo_sb = sg.tile([128, DC], F32, tag="bo")
            dist_acc = sg.tile([128, MT * NCH], F32, tag="dacc")
            scale_col = sg.tile([128, MT], F32, tag="scol")
            scale_row = sg.tile([1, SHARD], F32, tag="srow")

            nc.sync.dma_start(out=WoT_all, in_=WoT[:].rearrange("(c p) d -> p c d", p=128))
            nc.sync.dma_start(out=bo_sb, in_=bo_c[:])

            # ========== phase A: shard projections + AllGather ==========
            with (
                tc.tile_pool(name="pa", bufs=1) as pa,
                tc.tile_pool(name="pa_st", bufs=3) as pa_st,
                tc.tile_pool(name="psum_a", bufs=3, space="PSUM") as ps_a,
            ):
                Hts_all = pa.tile([128, DC, SHARD], BF16, tag="hts")
                Wq_sb = pa.tile([128, DC, DIM], BF16, tag="wq")
                Wk_sb = pa.tile([128, DC, DIM], BF16, tag="wk")
                Wv_sb = pa.tile([128, DC, DIM], BF16, tag="wv")
                bq_sb = pa.tile([128, DC], F32, tag="bq")
                bk_sb = pa.tile([128, DC], F32, tag="bk")
                bv_bc = pa.tile([128, DIM], F32, tag="bvbc")

                nc.sync.dma_start(out=Hts_all, in_=Hts[:].rearrange("(c p) n -> p c n", p=128))
                nc.sync.dma_start(out=Wq_sb, in_=WqT[:].rearrange("(c p) d -> p c d", p=128))
                nc.sync.dma_start(out=bq_sb, in_=bq_c[:])
                nc.sync.dma_start(out=Wk_sb, in_=WkT[:].rearrange("(c p) d -> p c d", p=128))
                nc.sync.dma_start(out=Wv_sb, in_=WvT[:].rearrange("(c p) d -> p c d", p=128))
                nc.sync.dma_start(out=bk_sb, in_=bk_c[:])
                nc.sync.dma_start(out=bv_bc, in_=_bcast_rows(bv_r[:]))

                # Qt = Wq @ Hs^T + bq   -> Qt_all [d-chunk, m]
                for d in range(DC):
                    for mb in range(MBN):
                        q_ps = ps_a.tile([128, MBS], F32, tag="a")
                        for e in range(DC):
                            nc.tensor.matmul(
                                q_ps,
                                Wq_sb[:, e, d * 128:(d + 1) * 128],
                                Hts_all[:, e, mb * MBS:(mb + 1) * MBS],
                                start=(e == 0), stop=(e == DC - 1),
                            )
                        nc.scalar.activation(
                            Qt_all[:, d, mb * MBS:(mb + 1) * MBS], q_ps,
                            AF.Identity, bias=bq_sb[:, d:d + 1], scale=1.0,
                        )

                # Kt shard -> ktag_i; V shard -> vag_i
                for d in range(DC):
                    for mb in range(MBN):
                        k_ps = ps_a.tile([128, MBS], F32, tag="a")
                        for e in range(DC):
                            nc.tensor.matmul(
                                k_ps,
                                Wk_sb[:, e, d * 128:(d + 1) * 128],
                                Hts_all[:, e, mb * MBS:(mb + 1) * MBS],
                                start=(e == 0), stop=(e == DC - 1),
                            )
                        k_st = pa_st.tile([128, MBS], BF16, tag="kst")
                        nc.scalar.activation(
                            k_st, k_ps, AF.Identity,
                            bias=bk_sb[:, d:d + 1], scale=1.0,
                        )
                        nc.sync.dma_start(
                            out=ktag_i[d * 128:(d + 1) * 128,
                                       mb * MBS:(mb + 1) * MBS],
                            in_=k_st,
                        )
                for i in range(MT):
                    v_ps = ps_a.tile([128, DIM], F32, tag="a")
                    for e in range(DC):
                        nc.tensor.matmul(
                            v_ps,
                            Hts_all[:, e, i * 128:(i + 1) * 128],
                            Wv_sb[:, e, :],
                            start=(e == 0), stop=(e == DC - 1),
                        )
                    v_st = pa_st.tile([128, DIM], BF16, tag="vst")
                    nc.vector.tensor_tensor(v_st, v_ps, bv_bc, ALU.add)
                    nc.sync.dma_start(
                        out=vag_i[i * 128:(i + 1) * 128, :], in_=v_st)

                # AllGather full Kt and V (overlaps phase B)
                nc.gpsimd.collective_compute(
                    "AllGather", ALU.bypass,
                    replica_groups=[list(range(NCORES))],
                    ins=[ktag_i.opt()], outs=[ktag_o.opt()],
                )
                nc.gpsimd.collective_compute(
                    "AllGather", ALU.bypass,
                    replica_groups=[list(range(NCORES))],
                    ins=[vag_i.opt()], outs=[vag_o.opt()],
                )
                for r in range(NCORES):
                    nc.sync.dma_start(
                        out=Kt_all[:, :, r * SHARD:(r + 1) * SHARD],
                        in_=ktag_o[r].rearrange("(c p) n -> p c n", p=128),
                    )
                    nc.sync.dma_start(
                        out=V_all[:, r * MT:(r + 1) * MT, :],
                        in_=vag_o[r].rearrange("(t p) d -> p t d", p=128),
                    )

            # ================= phase B: distance scale ==================
            with (
                tc.tile_pool(name="pb", bufs=1) as pb,
                tc.tile_pool(name="pb_tt", bufs=3) as pb_tt,
                tc.tile_pool(name="pb_st", bufs=3) as pb_st,
                tc.tile_pool(name="psum_b", bufs=3, space="PSUM") as ps_b,
            ):
                Tts_all = pb.tile([128, DC, SHARD], BF16, tag="tts")
                tts_sb = pb.tile([128, MT], F32, tag="ttssc")
                nc.sync.dma_start(out=Tts_all, in_=Tts[:].rearrange("(c p) n -> p c n", p=128))
                nc.sync.dma_start(out=tts_sb, in_=tts[:])

                for nch in range(NCH):
                    b0 = nch * 512
                    tt_sl = pb_tt.tile([128, DC, 512], BF16, tag="ttsl")
                    nc.sync.dma_start(
                        out=tt_sl,
                        in_=Tt[:, b0:b0 + 512].rearrange("(c p) n -> p c n", p=128),
                    )
                    ttn_bc = pb_tt.tile([128, 512], F32, tag="ttnb")
                    nc.sync.dma_start(out=ttn_bc, in_=_bcast_rows(ttn[:, b0:b0 + 512]))
                    for mt in range(MT):
                        g_ps = ps_b.tile([128, 512], F32, tag="g")
                        for e in range(DC):
                            nc.tensor.matmul(
                                g_ps,
                                Tts_all[:, e, mt * 128:(mt + 1) * 128],
                                tt_sl[:, e, :],
                                start=(e == 0), stop=(e == DC - 1),
                            )
                        x_t = pb_st.tile([128, 512], F32, tag="x")
                        nc.vector.scalar_tensor_tensor(
                            x_t, g_ps, -2.0, ttn_bc,
                            op0=ALU.mult, op1=ALU.add,
                        )
                        nc.vector.tensor_scalar(
                            x_t, x_t, tts_sb[:, mt:mt + 1], 1e-8,
                            op0=ALU.add, op1=ALU.max,
                        )
                        d_t = pb_st.tile([128, 512], BF16, tag="dst")
                        nc.scalar.activation(
                            d_t, x_t, AF.Sqrt, bias=0.0, scale=1.0,
                            accum_out=dist_acc[:, mt * NCH + nch: mt * NCH + nch + 1],
                        )

                # scale_col = 1 / (1 + mean) ; bounce to row layout via DRAM
                ds_sum = pb_st.tile([128, MT], F32, tag="dsum")
                nc.vector.tensor_reduce(
                    ds_sum,
                    dist_acc[:].rearrange("p (m t) -> p m t", t=NCH),
                    axis=AX.X, op=ALU.add,
                )
                sc_t = pb_st.tile([128, MT], F32, tag="sct")
                nc.vector.tensor_scalar(
                    sc_t, ds_sum, 1.0 / N, 1.0, op0=ALU.mult, op1=ALU.add,
                )
                nc.vector.reciprocal(scale_col, sc_t)
                nc.sync.dma_start(out=sscr[:].rearrange("t p -> p t"), in_=scale_col)
                nc.sync.dma_start(out=scale_row, in_=sscr[:].rearrange("t p -> () (t p)"))

            # ================= phase C: attention + projection ==========
            with (
                tc.tile_pool(name="pc", bufs=1) as pc,
                tc.tile_pool(name="pc_pt", bufs=3) as pc_pt,
                tc.tile_pool(name="pc_st", bufs=3) as pc_st,
                tc.tile_pool(name="pc_den", bufs=2) as pc_den,
                tc.tile_pool(name="psum_o", bufs=4, space="PSUM") as ps_o,
                tc.tile_pool(name="psum_s", bufs=2, space="PSUM") as ps_w,
                tc.tile_pool(name="psum_pj", bufs=2, space="PSUM") as ps_pj,
            ):
                ones_sb = pc.tile([128, 1], F32, tag="ones")
                nc.vector.memset(ones_sb, 1.0)
                ot_sb = pc.tile([128, MBN, DC, MBS], BF16, tag="ot")
                os_bc = pc.tile([128, MBN, MBS], F32, tag="osbc")

                def emit_proj(mb):
                    """Out^T = (Wo @ O^T) * os + bo for one m-block."""
                    for e in range(DC):
                        p_ps = ps_pj.tile([128, MBS], F32, tag="pj",
                                          name=f"pj{mb}_{e}")
                        for dv in range(DC):
                            nc.tensor.matmul(
                                p_ps,
                                WoT_all[:, dv, e * 128:(e + 1) * 128],
                                ot_sb[:, mb, dv, :],
                                start=(dv == 0), stop=(dv == DC - 1),
                            )
                        sc_st = pc_st.tile([128, MBS], F32, tag="scst",
                                           name=f"scst{mb}_{e}")
                        nc.vector.tensor_tensor(sc_st, p_ps, os_bc[:, mb, :],
                                                ALU.mult)
                        out_st = pc_st.tile([128, MBS], F32, tag="outst",
                                            name=f"outst{mb}_{e}")
                        nc.scalar.activation(out_st, sc_st, AF.Identity,
                                             bias=bo_sb[:, e:e + 1], scale=1.0)
                        nc.sync.dma_start(
                            out=OutT[e * 128:(e + 1) * 128,
                                     mb * MBS:(mb + 1) * MBS],
                            in_=out_st,
                        )

                for mb in range(MBN):
                    o_ps = [ps_o.tile([128, MBS], F32, tag="o", name=f"ops{mb}_{i}")
                            for i in range(DC)]
                    den_acc = pc_den.tile([128, MBS], F32, tag="den",
                                          name=f"den{mb}")
                    pt_prev = None
                    for nt in range(NT):
                        s_ps = ps_w.tile([128, MBS], F32, tag="s")
                        for e in range(DC):
                            nc.tensor.matmul(
                                s_ps,
                                Kt_all[:, e, nt * 128:(nt + 1) * 128],
                                Qt_all[:, e, mb * MBS:(mb + 1) * MBS],
                                start=(e == 0), stop=(e == DC - 1),
                            )
                        # software pipeline: PV of previous tile first so PE
                        # never waits on the current exp
                        if pt_prev is not None:
                            pnt, ptile = pt_prev
                            for dv in range(DC):
                                nc.tensor.matmul(
                                    o_ps[dv],
                                    V_all[:, pnt, dv * 128:(dv + 1) * 128],
                                    ptile,
                                    start=(pnt == 0), stop=(pnt == NT - 1),
                                )
                        if nt == 2 and mb > 0:
                            emit_proj(mb - 1)  # overlaps previous block's tail
                        p_t = pc_pt.tile([128, MBS], BF16, tag="pt")
                        nc.scalar.activation(p_t, s_ps, AF.Exp, bias=0.0,
                                             scale=float(INV_SQRT_D))
                        # softmax denominator on DVE (keeps PE free)
                        if nt == 0:
                            nc.vector.tensor_copy(den_acc, p_t)
                        else:
                            nc.vector.tensor_tensor(
                                den_acc, den_acc, p_t, ALU.add)
                        pt_prev = (nt, p_t)
                    pnt, ptile = pt_prev
                    for dv in range(DC):
                        nc.tensor.matmul(
                            o_ps[dv],
                            V_all[:, pnt, dv * 128:(dv + 1) * 128],
                            ptile,
                            start=(pnt == 0), stop=(pnt == NT - 1),
                        )

                    # raw O^T copies (frees psum for the next m-block)
                    for dv in range(DC):
                        nc.vector.tensor_copy(ot_sb[:, mb, dv, :], o_ps[dv])

                    # os = scale_s/denom -> row -> broadcast via DRAM bounce
                    # (overlaps with the next attention pass / projections).
                    # Partition-reduce of the f32 denominator via a single
                    # fp32 ones-matmul (~1us on the PE).
                    den_ps = ps_pj.tile([1, MBS], F32, tag="pj",
                                        name=f"dps{mb}")
                    nc.tensor.matmul(den_ps, ones_sb, den_acc,
                                     start=True, stop=True)
                    rec_d = pc_st.tile([1, MBS], F32, tag="recd",
                                       name=f"recd{mb}")
                    nc.vector.reciprocal(rec_d, den_ps)
                    os_row = pc_st.tile([1, MBS], F32, tag="osr",
                                        name=f"osr{mb}")
                    nc.vector.tensor_tensor(
                        os_row, rec_d, scale_row[:, mb * MBS:(mb + 1) * MBS],
                        ALU.mult)
                    nc.sync.dma_start(out=oscr[mb:mb + 1, :], in_=os_row)
                    nc.sync.dma_start(out=os_bc[:, mb, :],
                                      in_=_bcast_rows(oscr[mb:mb + 1, :]))

                emit_proj(MBN - 1)

    nc.compile()
    return nc


def prepare_in_maps(H, T, Wq, bq, Wk, bk, Wv, bv, Wo, bo):
    H = np.asarray(H, np.float32)
    T = np.asarray(T, np.float32)
    Ht = np.ascontiguousarray(H.T).astype(bf16np)
    Tt = np.ascontiguousarray(T.T).astype(bf16np)
    tt = (T.astype(np.float64) ** 2).sum(axis=1).astype(np.float32)
    ttn_row = tt.reshape(1, N)

    def wT(W):
        return np.ascontiguousarray(np.asarray(W, np.float32).T).astype(bf16np)

    def bcol(b):
        return np.ascontiguousarray(
            np.asarray(b, np.float32).reshape(DC, 128).T)

    shared = {
        "Tt": Tt, "ttn": ttn_row,
        "WqT": wT(Wq), "WkT": wT(Wk), "WvT": wT(Wv), "WoT": wT(Wo),
        "bq_c": bcol(bq), "bk_c": bcol(bk), "bo_c": bcol(bo),
        "bv_r": np.asarray(bv, np.float32).reshape(1, DIM).copy(),
    }
    in_maps = []
    for c in range(NCORES):
        sl = slice(c * SHARD, (c + 1) * SHARD)
        in_maps.append({
            **shared,
            "Hts": np.ascontiguousarray(Ht[:, sl]),
            "Tts": np.ascontiguousarray(Tt[:, sl]),
            "tts": np.ascontiguousarray(
                tt[sl].reshape(MT, 128).T),
        })
    return in_maps


def run_on_hw(in_maps, trace=False):
    nc = build_kernel()
    res = bass_utils.run_bass_kernel_spmd(
        nc, in_maps, core_ids=list(range(NCORES)), trace=trace)
    return res


def kernel(H, T, Wq, bq, Wk, bk, Wv, bv, Wo, bo):
    in_maps = prepare_in_maps(H, T, Wq, bq, Wk, bk, Wv, bv, Wo, bo)
    res = run_on_hw(in_maps, trace=False)
    out = np.empty((N, DIM), np.float32)
    for c in range(NCORES):
        out[c * SHARD:(c + 1) * SHARD] = res.results[c]["OutT"].T
    return out


# revision 24
# speedup vs baseline: 1.0551x; 1.0551x over previous
"""Trainium2 Bass kernel for nn_CausalAttention_41961830482398.

Computes, for H,T [8192,512] and dim-512 linear layers Wq/Wk/Wv/Wo:
    dist  = pairwise_distances(T)                 # [N,N]
    scale = 1/(1 + mean(dist, axis=1))            # [N,1]
    Q,K,V = H@W{q,k,v}.T + b{q,k,v}
    attn  = softmax(Q@K.T / sqrt(512))
    out   = ((attn*scale) @ V) @ Wo.T + bo

Sharding: sequence-parallel over the row dim N across 8 cores (1024 rows
per core).  Each core computes only its own K/V shard; the full K/V are
assembled with an AllGather that overlaps the distance phase.
Everything is computed in a transposed ("S^T") layout so that no
on-device transposes are needed:

  phase A: Kt_s = Wk@Hs^T + bk [512,1024], V_s = Hs@Wv^T + bv
           [1024,512], Qt = Wq@Hs^T + bq [512,1024]; AllGather Kt/V.
  phase B: G = Ts@T^T, dist = sqrt(max(tts+ttn-2G,0)+1e-8),
           row-mean via ACT accum -> scale_s  [1024]
           (overlaps the K/V AllGather)
  phase C: S^T tile = K@Qs^T  [n=128, m=512]; Pt = exp(S^T/sqrt(d));
           O^T += V^T@P^T via lhsT=V tiles; softmax denominator
           accumulated on DVE + one fp32 ones-matmul partition-reduce;
           projection: Out^T = (Wo@O^T)*(scale_s/denom) + bo -> DRAM.

Host passes pre-transposed/bf16-cast inputs and per-core shard slices;
the kernel returns Out^T per core which the host gathers + transposes.
"""

import numpy as np
import ml_dtypes

import concourse.bass as bass
import concourse.mybir as mybir
import concourse.tile as tile
from concourse import bacc
from concourse import bass_utils

N, DIM = 8192, 512
NCORES = 8
SHARD = N // NCORES          # 1024 rows per core
DC = DIM // 128              # 4 contraction chunks
NT = N // 128                # 64 key tiles
MBS = 512                    # m free-dim block
MBN = SHARD // MBS           # 2 m-blocks
NCH = N // 512               # 16 n chunks of 512
MT = SHARD // 128            # 8 m tiles per core
INV_SQRT_D = 1.0 / np.sqrt(np.float32(DIM))

BF16 = mybir.dt.bfloat16
F32 = mybir.dt.float32
AF = mybir.ActivationFunctionType
ALU = mybir.AluOpType
AX = mybir.AxisListType

bf16np = ml_dtypes.bfloat16


def _bcast_rows(ap, p=128):
    """DRAM row [1, n] -> broadcast AP [[0,p], [1,n]] for DMA replication."""
    return bass.AP(tensor=ap.tensor, offset=ap.offset, ap=[[0, p]] + ap.ap[1:])


def build_kernel():
    nc = bacc.Bacc("TRN2", target_bir_lowering=False, debug=False)

    # ---- DRAM I/O ------------------------------------------------------
    Hts = nc.dram_tensor("Hts", [DIM, SHARD], BF16, kind="ExternalInput")
    Tt = nc.dram_tensor("Tt", [DIM, N], BF16, kind="ExternalInput")
    Tts = nc.dram_tensor("Tts", [DIM, SHARD], BF16, kind="ExternalInput")
    ttn = nc.dram_tensor("ttn", [1, N], F32, kind="ExternalInput")
    tts = nc.dram_tensor("tts", [128, MT], F32, kind="ExternalInput")
    WqT = nc.dram_tensor("WqT", [DIM, DIM], BF16, kind="ExternalInput")
    WkT = nc.dram_tensor("WkT", [DIM, DIM], BF16, kind="ExternalInput")
    WvT = nc.dram_tensor("WvT", [DIM, DIM], BF16, kind="ExternalInput")
    WoT = nc.dram_tensor("WoT", [DIM, DIM], BF16, kind="ExternalInput")
    bq_c = nc.dram_tensor("bq_c", [128, DC], F32, kind="ExternalInput")
    bk_c = nc.dram_tensor("bk_c", [128, DC], F32, kind="ExternalInput")
    bo_c = nc.dram_tensor("bo_c", [128, DC], F32, kind="ExternalInput")
    bv_r = nc.dram_tensor("bv_r", [1, DIM], F32, kind="ExternalInput")
    OutT = nc.dram_tensor("OutT", [DIM, SHARD], F32, kind="ExternalOutput")

    with tile.TileContext(nc) as tc:
        with (
            tc.tile_pool(name="dram", bufs=1, space="DRAM") as dpool,
            tc.tile_pool(name="singles", bufs=1) as sg,
        ):
            sscr = dpool.tile([MT, 128], F32, tag="sscr")
            oscr = dpool.tile([MBN, MBS], F32, tag="oscr")
            # K/V AllGather in 4 interleaved pieces each, so phase C can
            # start consuming before the full gather completes
            PIECES = 4
            PW = SHARD // PIECES  # 256 columns/rows per piece
            ktp_i = [dpool.tile([DIM, PW], BF16, tag=f"ktpi{p}",
                                name=f"ktpi{p}") for p in range(PIECES)]
            ktp_o = [dpool.tile([NCORES, DIM, PW], BF16, tag=f"ktpo{p}",
                                name=f"ktpo{p}", addr_space="Shared")
                     for p in range(PIECES)]
            vp_i = [dpool.tile([PW, DIM], BF16, tag=f"vpi{p}",
                               name=f"vpi{p}") for p in range(PIECES)]
            vp_o = [dpool.tile([NCORES, PW, DIM], BF16, tag=f"vpo{p}",
                               name=f"vpo{p}", addr_space="Shared")
                    for p in range(PIECES)]

            # SBUF-resident for the whole kernel (KB/partition in comments)
            V_all = sg.tile([128, NT, DIM], BF16, tag="v")       # 64
            Kt_all = sg.tile([128, DC, N], BF16, tag="kt")       # 64
            Qt_all = sg.tile([128, DC, SHARD], BF16, tag="qt")   # 8
            WoT_all = sg.tile([128, DC, DIM], BF16, tag="wot")   # 4
            bo_sb = sg.tile([128, DC], F32, tag="bo")
            dist_acc = sg.tile([128, MT * NCH], F32, tag="dacc")
            scale_col = sg.tile([128, MT], F32, tag="scol")
            scale_row = sg.tile([1, SHARD], F32, tag="srow")

            nc.sync.dma_start(out=WoT_all, in_=WoT[:].rearrange("(c p) d -> p c d", p=128))
            nc.sync.dma_start(out=bo_sb, in_=bo_c[:])

            # ========== phase A: shard projections + AllGather ==========
            with (
                tc.tile_pool(name="pa", bufs=1) as pa,
                tc.tile_pool(name="pa_st", bufs=3) as pa_st,
                tc.tile_pool(name="psum_a", bufs=3, space="PSUM") as ps_a,
            ):
                Hts_all = pa.tile([128, DC, SHARD], BF16, tag="hts")
                Wq_sb = pa.tile([128, DC, DIM], BF16, tag="wq")
                Wk_sb = pa.tile([128, DC, DIM], BF16, tag="wk")
                Wv_sb = pa.tile([128, DC, DIM], BF16, tag="wv")
                bq_sb = pa.tile([128, DC], F32, tag="bq")
                bk_sb = pa.tile([128, DC], F32, tag="bk")
                bv_bc = pa.tile([128, DIM], F32, tag="bvbc")

                nc.sync.dma_start(out=Hts_all, in_=Hts[:].rearrange("(c p) n -> p c n", p=128))
                nc.sync.dma_start(out=Wq_sb, in_=WqT[:].rearrange("(c p) d -> p c d", p=128))
                nc.sync.dma_start(out=bq_sb, in_=bq_c[:])
                nc.sync.dma_start(out=Wk_sb, in_=WkT[:].rearrange("(c p) d -> p c d", p=128))
                nc.sync.dma_start(out=Wv_sb, in_=WvT[:].rearrange("(c p) d -> p c d", p=128))
                nc.sync.dma_start(out=bk_sb, in_=bk_c[:])
                nc.sync.dma_start(out=bv_bc, in_=_bcast_rows(bv_r[:]))

                # Qt = Wq @ Hs^T + bq   -> Qt_all [d-chunk, m]
                for d in range(DC):
                    for mb in range(MBN):
                        q_ps = ps_a.tile([128, MBS], F32, tag="a")
                        for e in range(DC):
                            nc.tensor.matmul(
                                q_ps,
                                Wq_sb[:, e, d * 128:(d + 1) * 128],
                                Hts_all[:, e, mb * MBS:(mb + 1) * MBS],
                                start=(e == 0), stop=(e == DC - 1),
                            )
                        nc.scalar.activation(
                            Qt_all[:, d, mb * MBS:(mb + 1) * MBS], q_ps,
                            AF.Identity, bias=bq_sb[:, d:d + 1], scale=1.0,
                        )

                # Kt shard -> ktag_i; V shard -> vag_i
                for d in range(DC):
                    for mb in range(MBN):
                        k_ps = ps_a.tile([128, MBS], F32, tag="a")
                        for e in range(DC):
                            nc.tensor.matmul(
                                k_ps,
                                Wk_sb[:, e, d * 128:(d + 1) * 128],
                                Hts_all[:, e, mb * MBS:(mb + 1) * MBS],
                                start=(e == 0), stop=(e == DC - 1),
                            )
                        k_st = pa_st.tile([128, MBS], BF16, tag="kst")
                        nc.scalar.activation(
                            k_st, k_ps, AF.Identity,
                            bias=bk_sb[:, d:d + 1], scale=1.0,
                        )
                        for h in range(2):
                            nc.sync.dma_start(
                                out=ktp_i[2 * mb + h][d * 128:(d + 1) * 128, :],
                                in_=k_st[:, h * PW:(h + 1) * PW],
                            )
                for i in range(MT):
                    v_ps = ps_a.tile([128, DIM], F32, tag="a")
                    for e in range(DC):
                        nc.tensor.matmul(
                            v_ps,
                            Hts_all[:, e, i * 128:(i + 1) * 128],
                            Wv_sb[:, e, :],
                            start=(e == 0), stop=(e == DC - 1),
                        )
                    v_st = pa_st.tile([128, DIM], BF16, tag="vst")
                    nc.vector.tensor_tensor(v_st, v_ps, bv_bc, ALU.add)
                    nc.sync.dma_start(
                        out=vp_i[i // 2][(i % 2) * 128:(i % 2 + 1) * 128, :],
                        in_=v_st)

                # piecewise AllGather of Kt and V (overlaps phase B); the
                # gather->SBUF loads go on the SWDGE queue so they never
                # block phase B's stream loads on the sync queue
                for p in range(PIECES):
                    nc.gpsimd.collective_compute(
                        "AllGather", ALU.bypass,
                        replica_groups=[list(range(NCORES))],
                        ins=[ktp_i[p].opt()], outs=[ktp_o[p].opt()],
                    )
                    for r in range(NCORES):
                        nc.gpsimd.dma_start(
                            out=Kt_all[:, :, r * SHARD + p * PW:
                                       r * SHARD + (p + 1) * PW],
                            in_=ktp_o[p][r].rearrange("(c q) n -> q c n", q=128),
                        )
                    nc.gpsimd.collective_compute(
                        "AllGather", ALU.bypass,
                        replica_groups=[list(range(NCORES))],
                        ins=[vp_i[p].opt()], outs=[vp_o[p].opt()],
                    )
                    for r in range(NCORES):
                        nc.gpsimd.dma_start(
                            out=V_all[:, r * MT + 2 * p: r * MT + 2 * p + 2, :],
                            in_=vp_o[p][r].rearrange("(t q) d -> q t d", q=128),
                        )

            # ================= phase B: distance scale ==================
            with (
                tc.tile_pool(name="pb", bufs=1) as pb,
                tc.tile_pool(name="pb_tt", bufs=3) as pb_tt,
                tc.tile_pool(name="pb_st", bufs=3) as pb_st,
                tc.tile_pool(name="psum_b", bufs=3, space="PSUM") as ps_b,
            ):
                Tts_all = pb.tile([128, DC, SHARD], BF16, tag="tts")
                tts_sb = pb.tile([128, MT], F32, tag="ttssc")
                nc.sync.dma_start(out=Tts_all, in_=Tts[:].rearrange("(c p) n -> p c n", p=128))
                nc.sync.dma_start(out=tts_sb, in_=tts[:])

                for nch in range(NCH):
                    b0 = nch * 512
                    tt_sl = pb_tt.tile([128, DC, 512], BF16, tag="ttsl")
                    nc.sync.dma_start(
                        out=tt_sl,
                        in_=Tt[:, b0:b0 + 512].rearrange("(c p) n -> p c n", p=128),
                    )
                    ttn_bc = pb_tt.tile([128, 512], F32, tag="ttnb")
                    nc.sync.dma_start(out=ttn_bc, in_=_bcast_rows(ttn[:, b0:b0 + 512]))
                    for mt in range(MT):
                        g_ps = ps_b.tile([128, 512], F32, tag="g")
                        for e in range(DC):
                            nc.tensor.matmul(
                                g_ps,
                                Tts_all[:, e, mt * 128:(mt + 1) * 128],
                                tt_sl[:, e, :],
                                start=(e == 0), stop=(e == DC - 1),
                            )
                        x_t = pb_st.tile([128, 512], F32, tag="x")
                        nc.vector.scalar_tensor_tensor(
                            x_t, g_ps, -2.0, ttn_bc,
                            op0=ALU.mult, op1=ALU.add,
                        )
                        nc.vector.tensor_scalar(
                            x_t, x_t, tts_sb[:, mt:mt + 1], 1e-8,
                            op0=ALU.add, op1=ALU.max,
                        )
                        d_t = pb_st.tile([128, 512], BF16, tag="dst")
                        nc.scalar.activation(
                            d_t, x_t, AF.Sqrt, bias=0.0, scale=1.0,
                            accum_out=dist_acc[:, mt * NCH + nch: mt * NCH + nch + 1],
                        )

                # scale_col = 1 / (1 + mean) ; bounce to row layout via DRAM
                ds_sum = pb_st.tile([128, MT], F32, tag="dsum")
                nc.vector.tensor_reduce(
                    ds_sum,
                    dist_acc[:].rearrange("p (m t) -> p m t", t=NCH),
                    axis=AX.X, op=ALU.add,
                )
                sc_t = pb_st.tile([128, MT], F32, tag="sct")
                nc.vector.tensor_scalar(
                    sc_t, ds_sum, 1.0 / N, 1.0, op0=ALU.mult, op1=ALU.add,
                )
                nc.vector.reciprocal(scale_col, sc_t)
                nc.sync.dma_start(out=sscr[:].rearrange("t p -> p t"), in_=scale_col)
                nc.sync.dma_start(out=scale_row, in_=sscr[:].rearrange("t p -> () (t p)"))

            # ================= phase C: attention + projection ==========
            with (
                tc.tile_pool(name="pc", bufs=1) as pc,
                tc.tile_pool(name="pc_pt", bufs=3) as pc_pt,
                tc.tile_pool(name="pc_st", bufs=3) as pc_st,
                tc.tile_pool(name="pc_den", bufs=2) as pc_den,
                tc.tile_pool(name="psum_o", bufs=4, space="PSUM") as ps_o,
                tc.tile_pool(name="psum_s", bufs=2, space="PSUM") as ps_w,
                tc.tile_pool(name="psum_pj", bufs=2, space="PSUM") as ps_pj,
            ):
                ones_sb = pc.tile([128, 1], F32, tag="ones")
                nc.vector.memset(ones_sb, 1.0)
                ot_sb = pc.tile([128, MBN, DC, MBS], BF16, tag="ot")
                os_bc = pc.tile([128, MBN, MBS], F32, tag="osbc")

                def emit_proj(mb):
                    """Out^T = (Wo @ O^T) * os + bo for one m-block."""
                    for e in range(DC):
                        p_ps = ps_pj.tile([128, MBS], F32, tag="pj",
                                          name=f"pj{mb}_{e}")
                        for dv in range(DC):
                            nc.tensor.matmul(
                                p_ps,
                                WoT_all[:, dv, e * 128:(e + 1) * 128],
                                ot_sb[:, mb, dv, :],
                                start=(dv == 0), stop=(dv == DC - 1),
                            )
                        sc_st = pc_st.tile([128, MBS], F32, tag="scst",
                                           name=f"scst{mb}_{e}")
                        nc.vector.tensor_tensor(sc_st, p_ps, os_bc[:, mb, :],
                                                ALU.mult)
                        out_st = pc_st.tile([128, MBS], F32, tag="outst",
                                            name=f"outst{mb}_{e}")
                        nc.scalar.activation(out_st, sc_st, AF.Identity,
                                             bias=bo_sb[:, e:e + 1], scale=1.0)
                        nc.sync.dma_start(
                            out=OutT[e * 128:(e + 1) * 128,
                                     mb * MBS:(mb + 1) * MBS],
                            in_=out_st,
                        )

                # consume key tiles in AllGather-piece order so the first
                # attention iterations only need the earliest pieces
                nts = [r * MT + t
                       for p in range(PIECES)
                       for r in range(NCORES)
                       for t in (2 * p, 2 * p + 1)]

                for mb in range(MBN):
                    o_ps = [ps_o.tile([128, MBS], F32, tag="o", name=f"ops{mb}_{i}")
                            for i in range(DC)]
                    den_acc = pc_den.tile([128, MBS], F32, tag="den",
                                          name=f"den{mb}")
                    pt_prev = None

                    def emit_pv(pidx, pnt, ptile, o_ps=o_ps):
                        for dv in range(DC):
                            nc.tensor.matmul(
                                o_ps[dv],
                                V_all[:, pnt, dv * 128:(dv + 1) * 128],
                                ptile,
                                start=(pidx == 0), stop=(pidx == NT - 1),
                            )

                    for idx, nt in enumerate(nts):
                        s_ps = ps_w.tile([128, MBS], F32, tag="s")
                        for e in range(DC):
                            nc.tensor.matmul(
                                s_ps,
                                Kt_all[:, e, nt * 128:(nt + 1) * 128],
                                Qt_all[:, e, mb * MBS:(mb + 1) * MBS],
                                start=(e == 0), stop=(e == DC - 1),
                            )
                        # software pipeline: PV of previous tile first so PE
                        # never waits on the current exp
                        if pt_prev is not None:
                            emit_pv(*pt_prev)
                        if idx == 2 and mb > 0:
                            emit_proj(mb - 1)  # overlaps previous block's tail
                        p_t = pc_pt.tile([128, MBS], BF16, tag="pt")
                        nc.scalar.activation(p_t, s_ps, AF.Exp, bias=0.0,
                                             scale=float(INV_SQRT_D))
                        # softmax denominator on DVE (keeps PE free)
                        if idx == 0:
                            nc.vector.tensor_copy(den_acc, p_t)
                        else:
                            nc.vector.tensor_tensor(
                                den_acc, den_acc, p_t, ALU.add)
                        pt_prev = (idx, nt, p_t)
                    emit_pv(*pt_prev)

                    # raw O^T copies (frees psum for the next m-block)
                    for dv in range(DC):
                        nc.vector.tensor_copy(ot_sb[:, mb, dv, :], o_ps[dv])

                    # os = scale_s/denom -> row -> broadcast via DRAM bounce
                    # (overlaps with the next attention pass / projections).
                    # Partition-reduce of the f32 denominator via a single
                    # fp32 ones-matmul (~1us on the PE).
                    den_ps = ps_pj.tile([1, MBS], F32, tag="pj",
                                        name=f"dps{mb}")
                    nc.tensor.matmul(den_ps, ones_sb, den_acc,
                                     start=True, stop=True)
                    rec_d = pc_st.tile([1, MBS], F32, tag="recd",
                                       name=f"recd{mb}")
                    nc.vector.reciprocal(rec_d, den_ps)
                    os_row = pc_st.tile([1, MBS], F32, tag="osr",
                                        name=f"osr{mb}")
                    nc.vector.tensor_tensor(
                        os_row, rec_d, scale_row[:, mb * MBS:(mb + 1) * MBS],
                        ALU.mult)
                    nc.sync.dma_start(out=oscr[mb:mb + 1, :], in_=os_row)
                    nc.sync.dma_start(out=os_bc[:, mb, :],
                                      in_=_bcast_rows(oscr[mb:mb + 1, :]))

                emit_proj(MBN - 1)

    nc.compile()
    return nc


def prepare_in_maps(H, T, Wq, bq, Wk, bk, Wv, bv, Wo, bo):
    H = np.asarray(H, np.float32)
    T = np.asarray(T, np.float32)
    Ht = np.ascontiguousarray(H.T).astype(bf16np)
    Tt = np.ascontiguousarray(T.T).astype(bf16np)
    tt = (T.astype(np.float64) ** 2).sum(axis=1).astype(np.float32)
    ttn_row = tt.reshape(1, N)

    def wT(W):
        return np.ascontiguousarray(np.asarray(W, np.float32).T).astype(bf16np)

    def bcol(b):
        return np.ascontiguousarray(
            np.asarray(b, np.float32).reshape(DC, 128).T)

    shared = {
        "Tt": Tt, "ttn": ttn_row,
        "WqT": wT(Wq), "WkT": wT(Wk), "WvT": wT(Wv), "WoT": wT(Wo),
        "bq_c": bcol(bq), "bk_c": bcol(bk), "bo_c": bcol(bo),
        "bv_r": np.asarray(bv, np.float32).reshape(1, DIM).copy(),
    }
    in_maps = []
    for c in range(NCORES):
        sl = slice(c * SHARD, (c + 1) * SHARD)
        in_maps.append({
            **shared,
            "Hts": np.ascontiguousarray(Ht[:, sl]),
            "Tts": np.ascontiguousarray(Tt[:, sl]),
            "tts": np.ascontiguousarray(
                tt[sl].reshape(MT, 128).T),
        })
    return in_maps


def run_on_hw(in_maps, trace=False):
    nc = build_kernel()
    res = bass_utils.run_bass_kernel_spmd(
        nc, in_maps, core_ids=list(range(NCORES)), trace=trace)
    return res


def kernel(H, T, Wq, bq, Wk, bk, Wv, bv, Wo, bo):
    in_maps = prepare_in_maps(H, T, Wq, bq, Wk, bk, Wv, bv, Wo, bo)
    res = run_on_hw(in_maps, trace=False)
    out = np.empty((N, DIM), np.float32)
    for c in range(NCORES):
        out[c * SHARD:(c + 1) * SHARD] = res.results[c]["OutT"].T
    return out


# revision 32
# speedup vs baseline: 1.1166x; 1.0583x over previous
"""Trainium2 Bass kernel for nn_CausalAttention_41961830482398.

Computes, for H,T [8192,512] and dim-512 linear layers Wq/Wk/Wv/Wo:
    dist  = pairwise_distances(T)                 # [N,N]
    scale = 1/(1 + mean(dist, axis=1))            # [N,1]
    Q,K,V = H@W{q,k,v}.T + b{q,k,v}
    attn  = softmax(Q@K.T / sqrt(512))
    out   = ((attn*scale) @ V) @ Wo.T + bo

Sharding: sequence-parallel over the row dim N across 8 cores (1024 rows
per core).  Each core computes only its own K/V shard; the full K/V are
assembled with an AllGather that overlaps the distance phase.
Everything is computed in a transposed ("S^T") layout so that no
on-device transposes are needed:

  phase A: Kt_s = Wk@Hs^T + bk [512,1024], V_s = Hs@Wv^T + bv
           [1024,512], Qt = Wq@Hs^T + bq [512,1024]; AllGather Kt/V.
  phase B: G = Ts@T^T, dist = sqrt(max(tts+ttn-2G,0)+1e-8),
           row-mean via ACT accum -> scale_s  [1024]
           (overlaps the K/V AllGather)
  phase C: S^T tile = K@Qs^T  [n=128, m=512]; Pt = exp(S^T/sqrt(d));
           O^T += V^T@P^T via lhsT=V tiles; softmax denominator
           accumulated on DVE + one fp32 ones-matmul partition-reduce;
           projection: Out^T = (Wo@O^T)*(scale_s/denom) + bo -> DRAM.

Host passes pre-transposed/bf16-cast inputs and per-core shard slices;
the kernel returns Out^T per core which the host gathers + transposes.
"""

import numpy as np
import ml_dtypes

import concourse.bass as bass
import concourse.mybir as mybir
import concourse.tile as tile
from concourse import bacc
from concourse import bass_utils

N, DIM = 8192, 512
NCORES = 8
SHARD = N // NCORES          # 1024 rows per core
DC = DIM // 128              # 4 contraction chunks
NT = N // 128                # 64 key tiles
MBS = 512                    # m free-dim block
MBN = SHARD // MBS           # 2 m-blocks
NCH = N // 512               # 16 n chunks of 512
MT = SHARD // 128            # 8 m tiles per core
INV_SQRT_D = 1.0 / np.sqrt(np.float32(DIM))

BF16 = mybir.dt.bfloat16
F32 = mybir.dt.float32
AF = mybir.ActivationFunctionType
ALU = mybir.AluOpType
AX = mybir.AxisListType

bf16np = ml_dtypes.bfloat16


def _bcast_rows(ap, p=128):
    """DRAM row [1, n] -> broadcast AP [[0,p], [1,n]] for DMA replication."""
    return bass.AP(tensor=ap.tensor, offset=ap.offset, ap=[[0, p]] + ap.ap[1:])


def build_kernel():
    nc = bacc.Bacc("TRN2", target_bir_lowering=False, debug=False)

    # ---- DRAM I/O ------------------------------------------------------
    Hts = nc.dram_tensor("Hts", [DIM, SHARD], BF16, kind="ExternalInput")
    Tt = nc.dram_tensor("Tt", [DIM, N], BF16, kind="ExternalInput")
    Tts = nc.dram_tensor("Tts", [DIM, SHARD], BF16, kind="ExternalInput")
    ttn = nc.dram_tensor("ttn", [1, N], F32, kind="ExternalInput")
    tts = nc.dram_tensor("tts", [128, MT], F32, kind="ExternalInput")
    WqT = nc.dram_tensor("WqT", [DIM, DIM], BF16, kind="ExternalInput")
    WkT = nc.dram_tensor("WkT", [DIM, DIM], BF16, kind="ExternalInput")
    WvT = nc.dram_tensor("WvT", [DIM, DIM], BF16, kind="ExternalInput")
    WoT = nc.dram_tensor("WoT", [DIM, DIM], BF16, kind="ExternalInput")
    bq_c = nc.dram_tensor("bq_c", [128, DC], F32, kind="ExternalInput")
    bk_c = nc.dram_tensor("bk_c", [128, DC], F32, kind="ExternalInput")
    bo_c = nc.dram_tensor("bo_c", [128, DC], F32, kind="ExternalInput")
    bv_r = nc.dram_tensor("bv_r", [1, DIM], F32, kind="ExternalInput")
    OutT = nc.dram_tensor("OutT", [DIM, SHARD], F32, kind="ExternalOutput")

    with tile.TileContext(nc) as tc:
        with (
            tc.tile_pool(name="dram", bufs=1, space="DRAM") as dpool,
            tc.tile_pool(name="singles", bufs=1) as sg,
        ):
            sscr = dpool.tile([MT, 128], F32, tag="sscr")
            oscr = dpool.tile([MBN, MBS], F32, tag="oscr")
            # K/V AllGather in 4 interleaved pieces each, so phase C can
            # start consuming before the full gather completes
            PIECES = 4
            PW = SHARD // PIECES  # 256 columns/rows per piece
            ktp_i = [dpool.tile([DIM, PW], BF16, tag=f"ktpi{p}",
                                name=f"ktpi{p}") for p in range(PIECES)]
            ktp_o = [dpool.tile([NCORES, DIM, PW], BF16, tag=f"ktpo{p}",
                                name=f"ktpo{p}", addr_space="Shared")
                     for p in range(PIECES)]
            vp_i = [dpool.tile([PW, DIM], BF16, tag=f"vpi{p}",
                               name=f"vpi{p}") for p in range(PIECES)]
            vp_o = [dpool.tile([NCORES, PW, DIM], BF16, tag=f"vpo{p}",
                               name=f"vpo{p}", addr_space="Shared")
                    for p in range(PIECES)]

            # SBUF-resident for the whole kernel (KB/partition in comments)
            V_all = sg.tile([128, NT, DIM], BF16, tag="v")       # 64
            Kt_all = sg.tile([128, DC, N], BF16, tag="kt")       # 64
            Qt_all = sg.tile([128, DC, SHARD], BF16, tag="qt")   # 8
            WoT_all = sg.tile([128, DC, DIM], BF16, tag="wot")   # 4
            bo_sb = sg.tile([128, DC], F32, tag="bo")
            dist_acc = sg.tile([128, MT * NCH], F32, tag="dacc")
            scale_col = sg.tile([128, MT], F32, tag="scol")
            scale_row = sg.tile([1, SHARD], F32, tag="srow")

            # prefetch phase-B's shard operand early (arrives during A)
            Tts_all = sg.tile([128, DC, SHARD], BF16, tag="tts")
            nc.sync.dma_start(out=Tts_all, in_=Tts[:].rearrange("(c p) n -> p c n", p=128))

            # ========== phase A: shard projections + AllGather ==========
            with (
                tc.tile_pool(name="pa", bufs=1) as pa,
                tc.tile_pool(name="pa_st", bufs=3) as pa_st,
                tc.tile_pool(name="psum_a", bufs=3, space="PSUM") as ps_a,
            ):
                Hts_all = pa.tile([128, DC, SHARD], BF16, tag="hts")
                Wq_sb = pa.tile([128, DC, DIM], BF16, tag="wq")
                Wk_sb = pa.tile([128, DC, DIM], BF16, tag="wk")
                Wv_sb = pa.tile([128, DC, DIM], BF16, tag="wv")
                bq_sb = pa.tile([128, DC], F32, tag="bq")
                bk_sb = pa.tile([128, DC], F32, tag="bk")
                bv_bc = pa.tile([128, DIM], F32, tag="bvbc")

                # critical startup loads first so the first Qt matmul can
                # start as early as possible
                nc.sync.dma_start(out=Hts_all, in_=Hts[:].rearrange("(c p) n -> p c n", p=128))
                nc.sync.dma_start(out=Wq_sb, in_=WqT[:].rearrange("(c p) d -> p c d", p=128))
                nc.sync.dma_start(out=bq_sb, in_=bq_c[:])
                nc.sync.dma_start(out=Wk_sb, in_=WkT[:].rearrange("(c p) d -> p c d", p=128))
                nc.sync.dma_start(out=Wv_sb, in_=WvT[:].rearrange("(c p) d -> p c d", p=128))
                nc.sync.dma_start(out=bk_sb, in_=bk_c[:])
                nc.sync.dma_start(out=bv_bc, in_=_bcast_rows(bv_r[:]))
                nc.sync.dma_start(out=WoT_all, in_=WoT[:].rearrange("(c p) d -> p c d", p=128))
                nc.sync.dma_start(out=bo_sb, in_=bo_c[:])

                # Qt = Wq @ Hs^T + bq   -> Qt_all [d-chunk, m]
                for d in range(DC):
                    for mb in range(MBN):
                        q_ps = ps_a.tile([128, MBS], F32, tag="a")
                        for e in range(DC):
                            nc.tensor.matmul(
                                q_ps,
                                Wq_sb[:, e, d * 128:(d + 1) * 128],
                                Hts_all[:, e, mb * MBS:(mb + 1) * MBS],
                                start=(e == 0), stop=(e == DC - 1),
                            )
                        nc.scalar.activation(
                            Qt_all[:, d, mb * MBS:(mb + 1) * MBS], q_ps,
                            AF.Identity, bias=bq_sb[:, d:d + 1], scale=1.0,
                        )

                # Kt shard -> ktag_i; V shard -> vag_i
                for d in range(DC):
                    for mb in range(MBN):
                        k_ps = ps_a.tile([128, MBS], F32, tag="a")
                        for e in range(DC):
                            nc.tensor.matmul(
                                k_ps,
                                Wk_sb[:, e, d * 128:(d + 1) * 128],
                                Hts_all[:, e, mb * MBS:(mb + 1) * MBS],
                                start=(e == 0), stop=(e == DC - 1),
                            )
                        k_st = pa_st.tile([128, MBS], BF16, tag="kst")
                        nc.scalar.activation(
                            k_st, k_ps, AF.Identity,
                            bias=bk_sb[:, d:d + 1], scale=1.0,
                        )
                        for h in range(2):
                            nc.sync.dma_start(
                                out=ktp_i[2 * mb + h][d * 128:(d + 1) * 128, :],
                                in_=k_st[:, h * PW:(h + 1) * PW],
                            )
                for i in range(MT):
                    v_ps = ps_a.tile([128, DIM], F32, tag="a")
                    for e in range(DC):
                        nc.tensor.matmul(
                            v_ps,
                            Hts_all[:, e, i * 128:(i + 1) * 128],
                            Wv_sb[:, e, :],
                            start=(e == 0), stop=(e == DC - 1),
                        )
                    v_st = pa_st.tile([128, DIM], BF16, tag="vst")
                    nc.vector.tensor_tensor(v_st, v_ps, bv_bc, ALU.add)
                    nc.sync.dma_start(
                        out=vp_i[i // 2][(i % 2) * 128:(i % 2 + 1) * 128, :],
                        in_=v_st)

                # piecewise AllGather of Kt and V (overlaps phase B); the
                # gather->SBUF loads go on the SWDGE queue so they never
                # block phase B's stream loads on the sync queue
                for p in range(PIECES):
                    nc.gpsimd.collective_compute(
                        "AllGather", ALU.bypass,
                        replica_groups=[list(range(NCORES))],
                        ins=[ktp_i[p].opt()], outs=[ktp_o[p].opt()],
                    )
                    for r in range(NCORES):
                        nc.gpsimd.dma_start(
                            out=Kt_all[:, :, r * SHARD + p * PW:
                                       r * SHARD + (p + 1) * PW],
                            in_=ktp_o[p][r].rearrange("(c q) n -> q c n", q=128),
                        )
                    nc.gpsimd.collective_compute(
                        "AllGather", ALU.bypass,
                        replica_groups=[list(range(NCORES))],
                        ins=[vp_i[p].opt()], outs=[vp_o[p].opt()],
                    )
                    for r in range(NCORES):
                        nc.gpsimd.dma_start(
                            out=V_all[:, r * MT + 2 * p: r * MT + 2 * p + 2, :],
                            in_=vp_o[p][r].rearrange("(t q) d -> q t d", q=128),
                        )

            # ================= phase B: distance scale ==================
            with (
                tc.tile_pool(name="pb", bufs=1) as pb,
                tc.tile_pool(name="pb_tt", bufs=4) as pb_tt,
                tc.tile_pool(name="pb_st", bufs=3) as pb_st,
                tc.tile_pool(name="psum_b", bufs=3, space="PSUM") as ps_b,
            ):
                tts_sb = pb.tile([128, MT], F32, tag="ttssc")
                nc.sync.dma_start(out=tts_sb, in_=tts[:])

                for nch in range(NCH):
                    b0 = nch * 512
                    tt_sl = pb_tt.tile([128, DC, 512], BF16, tag="ttsl")
                    nc.sync.dma_start(
                        out=tt_sl,
                        in_=Tt[:, b0:b0 + 512].rearrange("(c p) n -> p c n", p=128),
                    )
                    ttn_bc = pb_tt.tile([128, 512], F32, tag="ttnb")
                    nc.sync.dma_start(out=ttn_bc, in_=_bcast_rows(ttn[:, b0:b0 + 512]))
                    for mt in range(MT):
                        g_ps = ps_b.tile([128, 512], F32, tag="g")
                        for e in range(DC):
                            nc.tensor.matmul(
                                g_ps,
                                Tts_all[:, e, mt * 128:(mt + 1) * 128],
                                tt_sl[:, e, :],
                                start=(e == 0), stop=(e == DC - 1),
                            )
                        x_t = pb_st.tile([128, 512], F32, tag="x")
                        nc.vector.scalar_tensor_tensor(
                            x_t, g_ps, -2.0, ttn_bc,
                            op0=ALU.mult, op1=ALU.add,
                        )
                        nc.vector.tensor_scalar(
                            x_t, x_t, tts_sb[:, mt:mt + 1], 1e-8,
                            op0=ALU.add, op1=ALU.max,
                        )
                        d_t = pb_st.tile([128, 512], BF16, tag="dst")
                        nc.scalar.activation(
                            d_t, x_t, AF.Sqrt, bias=0.0, scale=1.0,
                            accum_out=dist_acc[:, mt * NCH + nch: mt * NCH + nch + 1],
                        )

                # scale_col = 1 / (1 + mean) ; bounce to row layout via DRAM
                ds_sum = pb_st.tile([128, MT], F32, tag="dsum")
                nc.vector.tensor_reduce(
                    ds_sum,
                    dist_acc[:].rearrange("p (m t) -> p m t", t=NCH),
                    axis=AX.X, op=ALU.add,
                )
                sc_t = pb_st.tile([128, MT], F32, tag="sct")
                nc.vector.tensor_scalar(
                    sc_t, ds_sum, 1.0 / N, 1.0, op0=ALU.mult, op1=ALU.add,
                )
                nc.vector.reciprocal(scale_col, sc_t)
                nc.sync.dma_start(out=sscr[:].rearrange("t p -> p t"), in_=scale_col)
                nc.sync.dma_start(out=scale_row, in_=sscr[:].rearrange("t p -> () (t p)"))

            # ================= phase C: attention + projection ==========
            with (
                tc.tile_pool(name="pc", bufs=1) as pc,
                tc.tile_pool(name="pc_pt", bufs=3) as pc_pt,
                tc.tile_pool(name="pc_st", bufs=3) as pc_st,
                tc.tile_pool(name="pc_den", bufs=2) as pc_den,
                tc.tile_pool(name="psum_o", bufs=4, space="PSUM") as ps_o,
                tc.tile_pool(name="psum_s", bufs=2, space="PSUM") as ps_w,
                tc.tile_pool(name="psum_pj", bufs=2, space="PSUM") as ps_pj,
            ):
                ones_sb = pc.tile([128, 1], F32, tag="ones")
                nc.vector.memset(ones_sb, 1.0)
                ot_sb = pc.tile([128, MBN, DC, MBS], BF16, tag="ot")
                os_bc = pc.tile([128, MBN, MBS], F32, tag="osbc")

                def emit_proj(mb):
                    """Out^T = (Wo @ O^T) * os + bo for one m-block."""
                    for e in range(DC):
                        p_ps = ps_pj.tile([128, MBS], F32, tag="pj",
                                          name=f"pj{mb}_{e}")
                        for dv in range(DC):
                            nc.tensor.matmul(
                                p_ps,
                                WoT_all[:, dv, e * 128:(e + 1) * 128],
                                ot_sb[:, mb, dv, :],
                                start=(dv == 0), stop=(dv == DC - 1),
                            )
                        sc_st = pc_st.tile([128, MBS], F32, tag="scst",
                                           name=f"scst{mb}_{e}")
                        nc.vector.tensor_tensor(sc_st, p_ps, os_bc[:, mb, :],
                                                ALU.mult)
                        out_st = pc_st.tile([128, MBS], F32, tag="outst",
                                            name=f"outst{mb}_{e}")
                        nc.scalar.activation(out_st, sc_st, AF.Identity,
                                             bias=bo_sb[:, e:e + 1], scale=1.0)
                        nc.sync.dma_start(
                            out=OutT[e * 128:(e + 1) * 128,
                                     mb * MBS:(mb + 1) * MBS],
                            in_=out_st,
                        )

                # consume key tiles in AllGather-piece order so the first
                # attention iterations only need the earliest pieces
                nts = [r * MT + t
                       for p in range(PIECES)
                       for r in range(NCORES)
                       for t in (2 * p, 2 * p + 1)]

                for mb in range(MBN):
                    o_ps = [ps_o.tile([128, MBS], F32, tag="o", name=f"ops{mb}_{i}")
                            for i in range(DC)]
                    den_acc = pc_den.tile([128, MBS], F32, tag="den",
                                          name=f"den{mb}")
                    pt_prev = None

                    def emit_pv(pidx, pnt, ptile, o_ps=o_ps):
                        for dv in range(DC):
                            nc.tensor.matmul(
                                o_ps[dv],
                                V_all[:, pnt, dv * 128:(dv + 1) * 128],
                                ptile,
                                start=(pidx == 0), stop=(pidx == NT - 1),
                            )

                    for idx, nt in enumerate(nts):
                        s_ps = ps_w.tile([128, MBS], F32, tag="s")
                        for e in range(DC):
                            nc.tensor.matmul(
                                s_ps,
                                Kt_all[:, e, nt * 128:(nt + 1) * 128],
                                Qt_all[:, e, mb * MBS:(mb + 1) * MBS],
                                start=(e == 0), stop=(e == DC - 1),
                            )
                        # software pipeline: PV of previous tile first so PE
                        # never waits on the current exp
                        if pt_prev is not None:
                            emit_pv(*pt_prev)
                        if idx == 2 and mb > 0:
                            emit_proj(mb - 1)  # overlaps previous block's tail
                        p_t = pc_pt.tile([128, MBS], BF16, tag="pt")
                        nc.scalar.activation(p_t, s_ps, AF.Exp, bias=0.0,
                                             scale=float(INV_SQRT_D))
                        # softmax denominator on DVE (keeps PE free)
                        if idx == 0:
                            nc.vector.tensor_copy(den_acc, p_t)
                        else:
                            nc.vector.tensor_tensor(
                                den_acc, den_acc, p_t, ALU.add)
                        pt_prev = (idx, nt, p_t)
                    emit_pv(*pt_prev)

                    # Partition-reduce of the f32 denominator via a single
                    # fp32 ones-matmul (~1us on the PE); issued before the
                    # O^T copies so the os chain starts as early as possible.
                    den_ps = ps_pj.tile([1, MBS], F32, tag="pj",
                                        name=f"dps{mb}")
                    nc.tensor.matmul(den_ps, ones_sb, den_acc,
                                     start=True, stop=True)

                    # raw O^T copies (frees psum for the next m-block)
                    for dv in range(DC):
                        nc.vector.tensor_copy(ot_sb[:, mb, dv, :], o_ps[dv])

                    # For the last m-block, issue the projection matmuls
                    # immediately (into the freed "o" psum slots); their
                    # scale/bias stages run once os_bc is ready.
                    last_pj = None
                    if mb == MBN - 1:
                        last_pj = []
                        for e in range(DC):
                            p_ps = ps_o.tile([128, MBS], F32, tag="o",
                                             name=f"pjl_{e}")
                            for dv in range(DC):
                                nc.tensor.matmul(
                                    p_ps,
                                    WoT_all[:, dv, e * 128:(e + 1) * 128],
                                    ot_sb[:, mb, dv, :],
                                    start=(dv == 0), stop=(dv == DC - 1),
                                )
                            last_pj.append(p_ps)

                    # os = scale_s/denom -> row -> broadcast via DRAM bounce
                    rec_d = pc_st.tile([1, MBS], F32, tag="recd",
                                       name=f"recd{mb}")
                    nc.vector.reciprocal(rec_d, den_ps)
                    os_row = pc_st.tile([1, MBS], F32, tag="osr",
                                        name=f"osr{mb}")
                    nc.vector.tensor_tensor(
                        os_row, rec_d, scale_row[:, mb * MBS:(mb + 1) * MBS],
                        ALU.mult)
                    nc.sync.dma_start(out=oscr[mb:mb + 1, :], in_=os_row)
                    nc.sync.dma_start(out=os_bc[:, mb, :],
                                      in_=_bcast_rows(oscr[mb:mb + 1, :]))

                    if last_pj is not None:
                        for e, p_ps in enumerate(last_pj):
                            sc_st = pc_st.tile([128, MBS], F32, tag="scst",
                                               name=f"scstL_{e}")
                            nc.vector.tensor_tensor(
                                sc_st, p_ps, os_bc[:, mb, :], ALU.mult)
                            out_st = pc_st.tile([128, MBS], F32, tag="outst",
                                                name=f"outstL_{e}")
                            nc.scalar.activation(
                                out_st, sc_st, AF.Identity,
                                bias=bo_sb[:, e:e + 1], scale=1.0)
                            nc.sync.dma_start(
                                out=OutT[e * 128:(e + 1) * 128,
                                         mb * MBS:(mb + 1) * MBS],
                                in_=out_st,
                            )

    nc.compile()
    return nc


def prepare_in_maps(H, T, Wq, bq, Wk, bk, Wv, bv, Wo, bo):
    H = np.asarray(H, np.float32)
    T = np.asarray(T, np.float32)
    Ht = np.ascontiguousarray(H.T).astype(bf16np)
    Tt = np.ascontiguousarray(T.T).astype(bf16np)
    tt = (T.astype(np.float64) ** 2).sum(axis=1).astype(np.float32)
    ttn_row = tt.reshape(1, N)

    def wT(W):
        return np.ascontiguousarray(np.asarray(W, np.float32).T).astype(bf16np)

    def bcol(b):
        return np.ascontiguousarray(
            np.asarray(b, np.float32).reshape(DC, 128).T)

    shared = {
        "Tt": Tt, "ttn": ttn_row,
        "WqT": wT(Wq), "WkT": wT(Wk), "WvT": wT(Wv), "WoT": wT(Wo),
        "bq_c": bcol(bq), "bk_c": bcol(bk), "bo_c": bcol(bo),
        "bv_r": np.asarray(bv, np.float32).reshape(1, DIM).copy(),
    }
    in_maps = []
    for c in range(NCORES):
        sl = slice(c * SHARD, (c + 1) * SHARD)
        in_maps.append({
            **shared,
            "Hts": np.ascontiguousarray(Ht[:, sl]),
            "Tts": np.ascontiguousarray(Tt[:, sl]),
            "tts": np.ascontiguousarray(
                tt[sl].reshape(MT, 128).T),
        })
    return in_maps


def run_on_hw(in_maps, trace=False):
    nc = build_kernel()
    res = bass_utils.run_bass_kernel_spmd(
        nc, in_maps, core_ids=list(range(NCORES)), trace=trace)
    return res


def kernel(H, T, Wq, bq, Wk, bk, Wv, bv, Wo, bo):
    in_maps = prepare_in_maps(H, T, Wq, bq, Wk, bk, Wv, bv, Wo, bo)
    res = run_on_hw(in_maps, trace=False)
    out = np.empty((N, DIM), np.float32)
    for c in range(NCORES):
        out[c * SHARD:(c + 1) * SHARD] = res.results[c]["OutT"].T
    return out
